# revision 8
# baseline (speedup 1.0000x reference)
"""BondMatrixMessage kernel for 8 TRN2 NeuronCores.

messages[b,e,i] = sum_{k,j} bond_state[b,e,k] * W[k,i,j] * atom_state[b,src_e,j]

Strategy (data-parallel over batch, 4 batches/core):
  - src gather as one-hot matmul on PE (srcT2 = atom2^T @ onehotT)
  - bond broadcast via selector matmuls on PE (rep_c = sel_c @ bondT2)
  - op_c = srcT2 * rep_c elementwise (DVE)  [the per-edge outer product
    bond[e,k]*src[e,j] laid out [(k,j) partitions, e free]]
  - messagesT[i,e] += W2_c^T @ op_c accumulated over 32 (k,j)-chunks (PE)
All host-side work is layout-only (transpose / tile / one-hot scatter).
"""

import sys

sys.path.insert(0, "/opt/trn_rl_repo")

import numpy as np

import concourse.bacc as bacc
import concourse.tile as tile
from concourse import mybir
from concourse.bass_utils import run_bass_kernel_spmd

B, A, E, D, K = 32, 256, 512, 64, 64
NCORES = 8
BPC = B // NCORES          # batches per core
NCHUNK = (K * D) // 128    # 32 contraction chunks of 128
NG = NCHUNK // 2           # 16 chunk pairs

F32 = mybir.dt.float32
F32R = mybir.dt.float32r


def _build(repeat: int = 1, use_f32r: bool = True):
    nc = bacc.Bacc("TRN2", debug=False)
    DT = F32R if use_f32r else F32

    atom2_d = nc.dram_tensor("atom2", [BPC, A, 2 * D], DT, kind="ExternalInput").ap()
    bondT2_d = nc.dram_tensor("bondT2", [BPC, 2 * K, E], DT, kind="ExternalInput").ap()
    onehot_d = nc.dram_tensor("onehotT", [BPC, A, E], DT, kind="ExternalInput").ap()
    w2_d = nc.dram_tensor("w2", [NCHUNK, 128, D], DT, kind="ExternalInput").ap()
    sel_d = nc.dram_tensor("sel", [NG, 128, 128], DT, kind="ExternalInput").ap()
    out_d = nc.dram_tensor("msgT", [BPC, D, E], F32, kind="ExternalOutput").ap()

    with tile.TileContext(nc) as tc:
        with (
            tc.tile_pool(name="consts", bufs=1) as consts,
            tc.tile_pool(name="inp", bufs=2) as inp,
            tc.tile_pool(name="work", bufs=6) as work,
            tc.tile_pool(name="ps_src", bufs=1, space="PSUM") as ps_src,
            tc.tile_pool(name="ps_rep", bufs=5, space="PSUM") as ps_rep,
            tc.tile_pool(name="ps_out", bufs=2, space="PSUM") as ps_out,
        ):
            # constants: W2 chunks + selector tiles (loaded once)
            w2_t = []
            for c in range(NCHUNK):
                t = consts.tile([128, D], DT, tag=f"w2_{c}")
                nc.sync.dma_start(t[:], w2_d[c])
                w2_t.append(t)
            sel_t = []
            for g in range(NG):
                t = consts.tile([128, 128], DT, tag=f"sel_{g}")
                nc.sync.dma_start(t[:], sel_d[g])
                sel_t.append(t)

            def body(_=None):
                for b in range(BPC):
                    # per-batch inputs
                    at0 = inp.tile([128, 2 * D], DT, tag="at0")
                    nc.sync.dma_start(at0[:], atom2_d[b, 0:128, :])
                    at1 = inp.tile([128, 2 * D], DT, tag="at1")
                    nc.sync.dma_start(at1[:], atom2_d[b, 128:256, :])
                    bt2 = inp.tile([2 * K, E], DT, tag="bt2")
                    nc.sync.dma_start(bt2[:], bondT2_d[b])
                    oh0 = inp.tile([128, E], DT, tag="oh0")
                    nc.sync.dma_start(oh0[:], onehot_d[b, 0:128, :])
                    oh1 = inp.tile([128, E], DT, tag="oh1")
                    nc.sync.dma_start(oh1[:], onehot_d[b, 128:256, :])

                    # srcT2[p=(h,j), e] = src[e, j] (duplicated over h)
                    ps = ps_src.tile([128, E], F32, tag="ps_src")
                    nc.tensor.matmul(ps[:], at0[:], oh0[:],
                                     start=True, stop=False)
                    nc.tensor.matmul(ps[:], at1[:], oh1[:],
                                     start=False, stop=True)
                    srcT2 = work.tile([128, E], DT, tag="srcT2")
                    nc.scalar.activation(srcT2[:], ps[:],
                                         mybir.ActivationFunctionType.Copy)

                    mout = ps_out.tile([D, E], F32, tag="mout")

                    def do_rep(g):
                        # rep for chunks 2g (sel rows 0:64) / 2g+1 (64:128);
                        # the two row-group matmuls run concurrently on PE
                        rep_e = ps_rep.tile([128, E], F32, tag="rep")
                        nc.tensor.matmul(rep_e[:], sel_t[g][0:64, :],
                                         bt2[0:64, :],
                                         start=True, stop=True,
                                         tile_position=(0, 0))
                        rep_o = ps_rep.tile([128, E], F32, tag="rep")
                        nc.tensor.matmul(rep_o[:], sel_t[g][64:128, :],
                                         bt2[64:128, :],
                                         start=True, stop=True,
                                         tile_position=(64, 0))
                        return rep_e, rep_o

                    # software pipeline, depth 2: reps for pairs g+1, g+2 are
                    # in flight while the Hadamards/mains of pair g run, so
                    # PE never stalls on DVE/GPSIMD and vice versa
                    from collections import deque
                    fifo = deque()
                    fifo.append(do_rep(0))
                    fifo.append(do_rep(1))
                    for g in range(NG):
                        if g + 2 < NG:
                            fifo.append(do_rep(g + 2))
                        rep_e, rep_o = fifo.popleft()
                        for half, rep in ((0, rep_e), (1, rep_o)):
                            c = 2 * g + half
                            op_t = work.tile([128, E], DT, tag="op")
                            if c % 8 in (1, 4, 6):
                                # offload 12/32 Hadamards to GPSIMD (~2x
                                # slower than DVE; needs the rep staged to
                                # SBUF: no PSUM reads on Pool) with the
                                # staging copy on the mostly-idle ACT
                                rep_s = work.tile([128, E], F32, tag="repc")
                                nc.scalar.activation(
                                    rep_s[:], rep[:],
                                    mybir.ActivationFunctionType.Copy)
                                nc.gpsimd.tensor_mul(op_t[:], srcT2[:],
                                                     rep_s[:])
                            else:
                                nc.vector.tensor_mul(op_t[:], srcT2[:], rep[:])
                            nc.tensor.matmul(mout[:], w2_t[c][:],
                                             op_t[:],
                                             start=(c == 0), stop=(c == NCHUNK - 1))

                    res = work.tile([D, E], F32, tag="res")
                    nc.scalar.activation(res[:], mout[:],
                                         mybir.ActivationFunctionType.Copy)
                    nc.sync.dma_start(out_d[b], res[:])

            if repeat == 1:
                body()
            else:
                tc.For_i_unrolled(0, repeat, 1, body, max_unroll=2)

    nc.compile()
    return nc


_CACHE = {}


def _get_nc(repeat=1, use_f32r=True):
    key = (repeat, use_f32r)
    if key not in _CACHE:
        _CACHE[key] = _build(repeat, use_f32r)
    return _CACHE[key]


def _prep_core_inputs(atom_state, bond_state, connectivity):
    """Host-side layout prep for one core's batch slice (no float math)."""
    bpc = atom_state.shape[0]
    atom2 = np.concatenate([atom_state, atom_state], axis=2)          # [b,A,2D]
    atom2 = np.ascontiguousarray(atom2, dtype=np.float32)
    bT = np.swapaxes(bond_state, 1, 2)                                # [b,K,E]
    bondT2 = np.concatenate([bT, bT], axis=1)                         # [b,2K,E]
    bondT2 = np.ascontiguousarray(bondT2, dtype=np.float32)
    idx = connectivity[:, :, 0].astype(np.int64)                      # [b,E]
    onehotT = (idx[:, None, :] == np.arange(A)[None, :, None])
    onehotT = np.ascontiguousarray(onehotT, dtype=np.float32)         # [b,A,E]
    assert atom2.shape == (bpc, A, 2 * D)
    return atom2, bondT2, onehotT


def _shared_inputs(bond_transform):
    w2 = bond_transform.reshape(K, D, D).transpose(0, 2, 1).reshape(K * D, D)
    w2 = np.ascontiguousarray(w2.reshape(NCHUNK, 128, D), dtype=np.float32)
    sel = np.zeros((NG, 128, 128), dtype=np.float32)
    for g in range(NG):
        for h in range(2):
            # rows 0:64  -> chunk 2g   : k = 4g + h
            sel[g, 4 * g + h, h * D:(h + 1) * D] = 1.0
            # rows 64:128-> chunk 2g+1 : k = 4g + 2 + h
            sel[g, 64 + 4 * g + 2 + h, h * D:(h + 1) * D] = 1.0
    return w2, sel


def kernel(atom_state, bond_state, connectivity, bond_transform,
           repeat=1, use_f32r=True):
    atom_state = np.asarray(atom_state, dtype=np.float32)
    bond_state = np.asarray(bond_state, dtype=np.float32)
    connectivity = np.asarray(connectivity)
    bond_transform = np.asarray(bond_transform, dtype=np.float32)

    nc = _get_nc(repeat, use_f32r)
    w2, sel = _shared_inputs(bond_transform)

    in_maps = []
    for m in range(NCORES):
        sl = slice(m * BPC, (m + 1) * BPC)
        atom2, bondT2, onehotT = _prep_core_inputs(
            atom_state[sl], bond_state[sl], connectivity[sl])
        in_maps.append({
            "atom2": atom2,
            "bondT2": bondT2,
            "onehotT": onehotT,
            "w2": w2,
            "sel": sel,
        })

    res = run_bass_kernel_spmd(nc, in_maps, list(range(NCORES)))

    out = np.empty((B, E, D), dtype=np.float32)
    for m in range(NCORES):
        msgT = res.results[m]["msgT"]                                  # [BPC,D,E]
        out[m * BPC:(m + 1) * BPC] = np.swapaxes(msgT, 1, 2)
    return out


if __name__ == "__main__":
    rng = np.random.default_rng(0)
    atom = rng.standard_normal((B, A, D)).astype(np.float32)
    bond = rng.standard_normal((B, E, K)).astype(np.float32)
    conn = rng.integers(0, A, size=(B, E, 2)).astype(np.int64)
    bt = rng.standard_normal((K, D * D)).astype(np.float32) * 0.01
    out = kernel(atom, bond, conn, bt)
    print("out", out.shape, out.dtype, float(np.abs(out).max()))


# revision 9
# speedup vs baseline: 1.2888x; 1.2888x over previous
"""BondMatrixMessage kernel for 8 TRN2 NeuronCores.

messages[b,e,i] = sum_{k,j} bond_state[b,e,k] * W[k,i,j] * atom_state[b,src_e,j]

Strategy (data-parallel over batch, 4 batches/core):
  - src gather as one-hot matmul on PE (srcT2 = atom2^T @ onehotT)
  - bond broadcast via selector matmuls on PE (rep_c = sel_c @ bondT2)
  - op_c = srcT2 * rep_c elementwise (DVE)  [the per-edge outer product
    bond[e,k]*src[e,j] laid out [(k,j) partitions, e free]]
  - messagesT[i,e] += W2_c^T @ op_c accumulated over 32 (k,j)-chunks (PE)
All host-side work is layout-only (transpose / tile / one-hot scatter).
"""

import sys

sys.path.insert(0, "/opt/trn_rl_repo")

import numpy as np

import concourse.bacc as bacc
import concourse.tile as tile
from concourse import mybir
from concourse.bass_utils import run_bass_kernel_spmd

B, A, E, D, K = 32, 256, 512, 64, 64
NCORES = 8
BPC = B // NCORES          # batches per core
NCHUNK = (K * D) // 128    # 32 contraction chunks of 128
NG = NCHUNK // 2           # 16 chunk pairs

F32 = mybir.dt.float32
F32R = mybir.dt.float32r


def _build(repeat: int = 1, use_f32r: bool = True):
    nc = bacc.Bacc("TRN2", debug=False)
    DT = F32R if use_f32r else F32

    atom2_d = nc.dram_tensor("atom2", [BPC, A, 2 * D], DT, kind="ExternalInput").ap()
    bondT2_d = nc.dram_tensor("bondT2", [BPC, 2 * K, E], DT, kind="ExternalInput").ap()
    onehot_d = nc.dram_tensor("onehotT", [BPC, A, E], DT, kind="ExternalInput").ap()
    w2_d = nc.dram_tensor("w2", [NCHUNK, 128, D], DT, kind="ExternalInput").ap()
    sel_d = nc.dram_tensor("sel", [NG, 128, 128], DT, kind="ExternalInput").ap()
    out_d = nc.dram_tensor("msgT", [BPC, D, E], F32, kind="ExternalOutput").ap()

    with tile.TileContext(nc) as tc:
        with (
            tc.tile_pool(name="consts", bufs=1) as consts,
            tc.tile_pool(name="inp", bufs=2) as inp,
            tc.tile_pool(name="work", bufs=6) as work,
            tc.tile_pool(name="ps_src", bufs=1, space="PSUM") as ps_src,
            tc.tile_pool(name="ps_rep", bufs=5, space="PSUM") as ps_rep,
            tc.tile_pool(name="ps_out", bufs=2, space="PSUM") as ps_out,
        ):
            # constants: W2 chunks + selector tiles (loaded once)
            w2_t = []
            for c in range(NCHUNK):
                t = consts.tile([128, D], DT, tag=f"w2_{c}")
                nc.sync.dma_start(t[:], w2_d[c])
                w2_t.append(t)
            sel_t = []
            for g in range(NG):
                t = consts.tile([128, 128], DT, tag=f"sel_{g}")
                nc.sync.dma_start(t[:], sel_d[g])
                sel_t.append(t)

            def body(_=None):
                for b in range(BPC):
                    # per-batch inputs
                    at0 = inp.tile([128, 2 * D], DT, tag="at0")
                    nc.sync.dma_start(at0[:], atom2_d[b, 0:128, :])
                    at1 = inp.tile([128, 2 * D], DT, tag="at1")
                    nc.sync.dma_start(at1[:], atom2_d[b, 128:256, :])
                    bt2 = inp.tile([2 * K, E], DT, tag="bt2")
                    nc.sync.dma_start(bt2[:], bondT2_d[b])
                    oh0 = inp.tile([128, E], DT, tag="oh0")
                    nc.sync.dma_start(oh0[:], onehot_d[b, 0:128, :])
                    oh1 = inp.tile([128, E], DT, tag="oh1")
                    nc.sync.dma_start(oh1[:], onehot_d[b, 128:256, :])

                    # srcT2[p=(h,j), e] = src[e, j] (duplicated over h)
                    ps = ps_src.tile([128, E], F32, tag="ps_src")
                    nc.tensor.matmul(ps[:], at0[:], oh0[:],
                                     start=True, stop=False)
                    nc.tensor.matmul(ps[:], at1[:], oh1[:],
                                     start=False, stop=True)
                    srcT2 = work.tile([128, E], DT, tag="srcT2")
                    nc.scalar.activation(srcT2[:], ps[:],
                                         mybir.ActivationFunctionType.Copy)

                    mout = ps_out.tile([D, E], F32, tag="mout")

                    def do_rep(g):
                        # rep for chunks 2g (sel rows 0:64) / 2g+1 (64:128);
                        # the two row-group matmuls run concurrently on PE
                        rep_e = ps_rep.tile([128, E], F32, tag="rep")
                        nc.tensor.matmul(rep_e[:], sel_t[g][0:64, :],
                                         bt2[0:64, :],
                                         start=True, stop=True,
                                         tile_position=(0, 0))
                        rep_o = ps_rep.tile([128, E], F32, tag="rep")
                        nc.tensor.matmul(rep_o[:], sel_t[g][64:128, :],
                                         bt2[64:128, :],
                                         start=True, stop=True,
                                         tile_position=(64, 0))
                        return rep_e, rep_o

                    # software pipeline, depth 2: reps for pairs g+1, g+2 are
                    # in flight while the Hadamards/mains of pair g run, so
                    # PE never stalls on DVE/GPSIMD and vice versa
                    from collections import deque
                    fifo = deque()
                    fifo.append(do_rep(0))
                    fifo.append(do_rep(1))
                    for g in range(NG):
                        if g + 2 < NG:
                            fifo.append(do_rep(g + 2))
                        rep_e, rep_o = fifo.popleft()
                        for half, rep in ((0, rep_e), (1, rep_o)):
                            c = 2 * g + half
                            op_t = work.tile([128, E], DT, tag="op")
                            if c % 8 in (1, 4, 6):
                                # offload 12/32 Hadamards to GPSIMD (~2x
                                # slower than DVE; needs the rep staged to
                                # SBUF: no PSUM reads on Pool) with the
                                # staging copy on the mostly-idle ACT
                                rep_s = work.tile([128, E], F32, tag="repc")
                                nc.scalar.activation(
                                    rep_s[:], rep[:],
                                    mybir.ActivationFunctionType.Copy)
                                nc.gpsimd.tensor_mul(op_t[:], srcT2[:],
                                                     rep_s[:])
                            else:
                                nc.vector.tensor_mul(op_t[:], srcT2[:], rep[:])
                            nc.tensor.matmul(mout[:], w2_t[c][:],
                                             op_t[:],
                                             start=(c == 0), stop=(c == NCHUNK - 1))

                    res = work.tile([D, E], F32, tag="res")
                    nc.scalar.activation(res[:], mout[:],
                                         mybir.ActivationFunctionType.Copy)
                    nc.sync.dma_start(out_d[b], res[:])

            if repeat == 1:
                body()
            else:
                tc.For_i_unrolled(0, repeat, 1, body, max_unroll=1)

    nc.compile()
    return nc


_CACHE = {}


def _get_nc(repeat=1, use_f32r=True):
    key = (repeat, use_f32r)
    if key not in _CACHE:
        _CACHE[key] = _build(repeat, use_f32r)
    return _CACHE[key]


def _prep_core_inputs(atom_state, bond_state, connectivity):
    """Host-side layout prep for one core's batch slice (no float math)."""
    bpc = atom_state.shape[0]
    atom2 = np.concatenate([atom_state, atom_state], axis=2)          # [b,A,2D]
    atom2 = np.ascontiguousarray(atom2, dtype=np.float32)
    bT = np.swapaxes(bond_state, 1, 2)                                # [b,K,E]
    bondT2 = np.concatenate([bT, bT], axis=1)                         # [b,2K,E]
    bondT2 = np.ascontiguousarray(bondT2, dtype=np.float32)
    idx = connectivity[:, :, 0].astype(np.int64)                      # [b,E]
    onehotT = (idx[:, None, :] == np.arange(A)[None, :, None])
    onehotT = np.ascontiguousarray(onehotT, dtype=np.float32)         # [b,A,E]
    assert atom2.shape == (bpc, A, 2 * D)
    return atom2, bondT2, onehotT


def _shared_inputs(bond_transform):
    w2 = bond_transform.reshape(K, D, D).transpose(0, 2, 1).reshape(K * D, D)
    w2 = np.ascontiguousarray(w2.reshape(NCHUNK, 128, D), dtype=np.float32)
    sel = np.zeros((NG, 128, 128), dtype=np.float32)
    for g in range(NG):
        for h in range(2):
            # rows 0:64  -> chunk 2g   : k = 4g + h
            sel[g, 4 * g + h, h * D:(h + 1) * D] = 1.0
            # rows 64:128-> chunk 2g+1 : k = 4g + 2 + h
            sel[g, 64 + 4 * g + 2 + h, h * D:(h + 1) * D] = 1.0
    return w2, sel


def kernel(atom_state, bond_state, connectivity, bond_transform,
           repeat=1, use_f32r=True):
    atom_state = np.asarray(atom_state, dtype=np.float32)
    bond_state = np.asarray(bond_state, dtype=np.float32)
    connectivity = np.asarray(connectivity)
    bond_transform = np.asarray(bond_transform, dtype=np.float32)

    nc = _get_nc(repeat, use_f32r)
    w2, sel = _shared_inputs(bond_transform)

    in_maps = []
    for m in range(NCORES):
        sl = slice(m * BPC, (m + 1) * BPC)
        atom2, bondT2, onehotT = _prep_core_inputs(
            atom_state[sl], bond_state[sl], connectivity[sl])
        in_maps.append({
            "atom2": atom2,
            "bondT2": bondT2,
            "onehotT": onehotT,
            "w2": w2,
            "sel": sel,
        })

    res = run_bass_kernel_spmd(nc, in_maps, list(range(NCORES)))

    out = np.empty((B, E, D), dtype=np.float32)
    for m in range(NCORES):
        msgT = res.results[m]["msgT"]                                  # [BPC,D,E]
        out[m * BPC:(m + 1) * BPC] = np.swapaxes(msgT, 1, 2)
    return out


if __name__ == "__main__":
    rng = np.random.default_rng(0)
    atom = rng.standard_normal((B, A, D)).astype(np.float32)
    bond = rng.standard_normal((B, E, K)).astype(np.float32)
    conn = rng.integers(0, A, size=(B, E, 2)).astype(np.int64)
    bt = rng.standard_normal((K, D * D)).astype(np.float32) * 0.01
    out = kernel(atom, bond, conn, bt)
    print("out", out.shape, out.dtype, float(np.abs(out).max()))


# revision 12
# speedup vs baseline: 1.3541x; 1.0507x over previous
"""BondMatrixMessage kernel for 8 TRN2 NeuronCores.

messages[b,e,i] = sum_{k,j} bond_state[b,e,k] * W[k,i,j] * atom_state[b,src_e,j]

Strategy (data-parallel over batch, 4 batches/core):
  - src gather as one-hot matmul on PE (srcT2 = atom2^T @ onehotT)
  - bond broadcast via selector matmuls on PE (rep_c = sel_c @ bondT2)
  - op_c = srcT2 * rep_c elementwise (DVE)  [the per-edge outer product
    bond[e,k]*src[e,j] laid out [(k,j) partitions, e free]]
  - messagesT[i,e] += W2_c^T @ op_c accumulated over 32 (k,j)-chunks (PE)
All host-side work is layout-only (transpose / tile / one-hot scatter).
"""

import sys

sys.path.insert(0, "/opt/trn_rl_repo")

import numpy as np

import concourse.bacc as bacc
import concourse.tile as tile
from concourse import mybir
from concourse.bass_utils import run_bass_kernel_spmd

B, A, E, D, K = 32, 256, 512, 64, 64
NCORES = 8
BPC = B // NCORES          # batches per core
NCHUNK = (K * D) // 128    # 32 contraction chunks of 128
NG = NCHUNK // 2           # 16 chunk pairs

F32 = mybir.dt.float32
F32R = mybir.dt.float32r


def _build(repeat: int = 1, use_f32r: bool = True):
    nc = bacc.Bacc("TRN2", debug=False)
    DT = F32R if use_f32r else F32

    atom2_d = nc.dram_tensor("atom2", [BPC, A, 2 * D], DT, kind="ExternalInput").ap()
    bondT2_d = nc.dram_tensor("bondT2", [BPC, 2 * K, E], DT, kind="ExternalInput").ap()
    onehot_d = nc.dram_tensor("onehotT", [BPC, A, E], DT, kind="ExternalInput").ap()
    w2_d = nc.dram_tensor("w2", [NCHUNK, 128, D], DT, kind="ExternalInput").ap()
    sel_d = nc.dram_tensor("sel", [NG, 128, 128], DT, kind="ExternalInput").ap()
    out_d = nc.dram_tensor("msgT", [BPC, D, E], F32, kind="ExternalOutput").ap()

    with tile.TileContext(nc) as tc:
        with (
            tc.tile_pool(name="consts", bufs=1) as consts,
            tc.tile_pool(name="inp", bufs=2) as inp,
            tc.tile_pool(name="work", bufs=6) as work,
            tc.tile_pool(name="ps_rep", bufs=6, space="PSUM") as ps_rep,
            tc.tile_pool(name="ps_out", bufs=2, space="PSUM") as ps_out,
        ):
            # constants: W2 chunks + selector tiles (loaded once)
            w2_t = []
            for c in range(NCHUNK):
                t = consts.tile([128, D], DT, tag=f"w2_{c}")
                nc.sync.dma_start(t[:], w2_d[c])
                w2_t.append(t)
            sel_t = []
            for g in range(NG):
                t = consts.tile([128, 128], DT, tag=f"sel_{g}")
                nc.sync.dma_start(t[:], sel_d[g])
                sel_t.append(t)

            def start_batch(b):
                """DMA loads + src gather matmuls for one batch; returns
                per-batch state (srcT2, bt2, mout)."""
                at0 = inp.tile([128, 2 * D], DT, tag="at0")
                nc.sync.dma_start(at0[:], atom2_d[b, 0:128, :])
                at1 = inp.tile([128, 2 * D], DT, tag="at1")
                nc.sync.dma_start(at1[:], atom2_d[b, 128:256, :])
                bt2 = inp.tile([2 * K, E], DT, tag="bt2")
                nc.sync.dma_start(bt2[:], bondT2_d[b])
                oh0 = inp.tile([128, E], DT, tag="oh0")
                nc.sync.dma_start(oh0[:], onehot_d[b, 0:128, :])
                oh1 = inp.tile([128, E], DT, tag="oh1")
                nc.sync.dma_start(oh1[:], onehot_d[b, 128:256, :])

                # srcT2[p=(h,j), e] = src[e, j] (duplicated over h)
                ps = ps_rep.tile([128, E], F32, tag="rep")
                nc.tensor.matmul(ps[:], at0[:], oh0[:],
                                 start=True, stop=False)
                nc.tensor.matmul(ps[:], at1[:], oh1[:],
                                 start=False, stop=True)
                srcT2 = work.tile([128, E], DT, tag="srcT2")
                nc.scalar.activation(srcT2[:], ps[:],
                                     mybir.ActivationFunctionType.Copy)
                mout = ps_out.tile([D, E], F32, tag="mout")
                return srcT2, bt2, mout, b

            def do_rep(st, g):
                _, bt2, _, _ = st
                rep_e = ps_rep.tile([128, E], F32, tag="rep")
                nc.tensor.matmul(rep_e[:], sel_t[g][0:64, :],
                                 bt2[0:64, :],
                                 start=True, stop=True,
                                 tile_position=(0, 0))
                rep_o = ps_rep.tile([128, E], F32, tag="rep")
                nc.tensor.matmul(rep_o[:], sel_t[g][64:128, :],
                                 bt2[64:128, :],
                                 start=True, stop=True,
                                 tile_position=(64, 0))
                return rep_e, rep_o

            def do_pair(st, g, reps):
                srcT2, _, mout, _ = st
                rep_e, rep_o = reps
                for half, rep in ((0, rep_e), (1, rep_o)):
                    c = 2 * g + half
                    op_t = work.tile([128, E], DT, tag="op")
                    if c % 8 in (1, 4, 6):
                        # offload 12/32 Hadamards to GPSIMD (~2x slower
                        # than DVE; rep staged to SBUF via the idle ACT
                        # since Pool can't read PSUM)
                        rep_s = work.tile([128, E], F32, tag="repc")
                        nc.scalar.activation(
                            rep_s[:], rep[:],
                            mybir.ActivationFunctionType.Copy)
                        nc.gpsimd.tensor_mul(op_t[:], srcT2[:], rep_s[:])
                    else:
                        nc.vector.tensor_mul(op_t[:], srcT2[:], rep[:])
                    nc.tensor.matmul(mout[:], w2_t[c][:], op_t[:],
                                     start=(c == 0), stop=(c == NCHUNK - 1))

            def finish_batch(st):
                _, _, mout, b = st
                res = work.tile([D, E], F32, tag="res")
                nc.scalar.activation(res[:], mout[:],
                                     mybir.ActivationFunctionType.Copy)
                nc.sync.dma_start(out_d[b], res[:])

            def body(_=None):
                # two batches interleaved through the chunk pipeline:
                # amortizes ramp/drain and keeps every engine fed
                from collections import deque
                for b0 in range(0, BPC, 2):
                    sts = [start_batch(b0), start_batch(b0 + 1)]
                    fifo = deque()
                    for st in sts:
                        fifo.append((st, 0, do_rep(st, 0)))
                    for st in sts:
                        fifo.append((st, 1, do_rep(st, 1)))
                    for g in range(NG):
                        for st in sts:
                            if g + 2 < NG:
                                fifo.append((st, g + 2, do_rep(st, g + 2)))
                        for _ in range(2):
                            st, gg, reps = fifo.popleft()
                            do_pair(st, gg, reps)
                    for st in sts:
                        finish_batch(st)

            if repeat == 1:
                body()
            else:
                tc.For_i_unrolled(0, repeat, 1, body, max_unroll=1)

    nc.compile()
    return nc


_CACHE = {}


def _get_nc(repeat=1, use_f32r=True):
    key = (repeat, use_f32r)
    if key not in _CACHE:
        _CACHE[key] = _build(repeat, use_f32r)
    return _CACHE[key]


def _prep_core_inputs(atom_state, bond_state, connectivity):
    """Host-side layout prep for one core's batch slice (no float math)."""
    bpc = atom_state.shape[0]
    atom2 = np.concatenate([atom_state, atom_state], axis=2)          # [b,A,2D]
    atom2 = np.ascontiguousarray(atom2, dtype=np.float32)
    bT = np.swapaxes(bond_state, 1, 2)                                # [b,K,E]
    bondT2 = np.concatenate([bT, bT], axis=1)                         # [b,2K,E]
    bondT2 = np.ascontiguousarray(bondT2, dtype=np.float32)
    idx = connectivity[:, :, 0].astype(np.int64)                      # [b,E]
    onehotT = (idx[:, None, :] == np.arange(A)[None, :, None])
    onehotT = np.ascontiguousarray(onehotT, dtype=np.float32)         # [b,A,E]
    assert atom2.shape == (bpc, A, 2 * D)
    return atom2, bondT2, onehotT


def _shared_inputs(bond_transform):
    w2 = bond_transform.reshape(K, D, D).transpose(0, 2, 1).reshape(K * D, D)
    w2 = np.ascontiguousarray(w2.reshape(NCHUNK, 128, D), dtype=np.float32)
    sel = np.zeros((NG, 128, 128), dtype=np.float32)
    for g in range(NG):
        for h in range(2):
            # rows 0:64  -> chunk 2g   : k = 4g + h
            sel[g, 4 * g + h, h * D:(h + 1) * D] = 1.0
            # rows 64:128-> chunk 2g+1 : k = 4g + 2 + h
            sel[g, 64 + 4 * g + 2 + h, h * D:(h + 1) * D] = 1.0
    return w2, sel


def kernel(atom_state, bond_state, connectivity, bond_transform,
           repeat=1, use_f32r=True):
    atom_state = np.asarray(atom_state, dtype=np.float32)
    bond_state = np.asarray(bond_state, dtype=np.float32)
    connectivity = np.asarray(connectivity)
    bond_transform = np.asarray(bond_transform, dtype=np.float32)

    nc = _get_nc(repeat, use_f32r)
    w2, sel = _shared_inputs(bond_transform)

    in_maps = []
    for m in range(NCORES):
        sl = slice(m * BPC, (m + 1) * BPC)
        atom2, bondT2, onehotT = _prep_core_inputs(
            atom_state[sl], bond_state[sl], connectivity[sl])
        in_maps.append({
            "atom2": atom2,
            "bondT2": bondT2,
            "onehotT": onehotT,
            "w2": w2,
            "sel": sel,
        })

    res = run_bass_kernel_spmd(nc, in_maps, list(range(NCORES)))

    out = np.empty((B, E, D), dtype=np.float32)
    for m in range(NCORES):
        msgT = res.results[m]["msgT"]                                  # [BPC,D,E]
        out[m * BPC:(m + 1) * BPC] = np.swapaxes(msgT, 1, 2)
    return out


if __name__ == "__main__":
    rng = np.random.default_rng(0)
    atom = rng.standard_normal((B, A, D)).astype(np.float32)
    bond = rng.standard_normal((B, E, K)).astype(np.float32)
    conn = rng.integers(0, A, size=(B, E, 2)).astype(np.int64)
    bt = rng.standard_normal((K, D * D)).astype(np.float32) * 0.01
    out = kernel(atom, bond, conn, bt)
    print("out", out.shape, out.dtype, float(np.abs(out).max()))


# revision 14
# speedup vs baseline: 1.9206x; 1.4183x over previous
"""BondMatrixMessage kernel for 8 TRN2 NeuronCores.

messages[b,e,i] = sum_{k,j} bond_state[b,e,k] * W[k,i,j] * atom_state[b,src_e,j]

Strategy (data-parallel over batch, 4 batches/core):
  - src gather as one-hot matmul on PE (srcT2 = atom2^T @ onehotT)
  - bond broadcast via selector matmuls on PE (rep_c = sel_c @ bondT2)
  - op_c = srcT2 * rep_c elementwise (DVE)  [the per-edge outer product
    bond[e,k]*src[e,j] laid out [(k,j) partitions, e free]]
  - messagesT[i,e] += W2_c^T @ op_c accumulated over 32 (k,j)-chunks (PE)
All host-side work is layout-only (transpose / tile / one-hot scatter).
"""

import sys

sys.path.insert(0, "/opt/trn_rl_repo")

import numpy as np

import concourse.bacc as bacc
import concourse.tile as tile
from concourse import mybir
from concourse.bass_utils import run_bass_kernel_spmd

B, A, E, D, K = 32, 256, 512, 64, 64
NCORES = 8
BPC = B // NCORES          # batches per core
NCHUNK = (K * D) // 128    # 32 contraction chunks of 128
NG = NCHUNK // 2           # 16 chunk pairs

F32 = mybir.dt.float32
F32R = mybir.dt.float32r


def _build(repeat: int = 1, use_f32r: bool = True):
    nc = bacc.Bacc("TRN2", debug=False)
    DT = F32R if use_f32r else F32

    atom2_d = nc.dram_tensor("atom2", [BPC, A, 2 * D], DT, kind="ExternalInput").ap()
    bondT2_d = nc.dram_tensor("bondT2", [BPC, 2 * K, E], DT, kind="ExternalInput").ap()
    onehot_d = nc.dram_tensor("onehotT", [BPC, A, E], DT, kind="ExternalInput").ap()
    w2_d = nc.dram_tensor("w2", [NCHUNK, 128, D], DT, kind="ExternalInput").ap()
    sel_d = nc.dram_tensor("sel", [NG, 128, 128], DT, kind="ExternalInput").ap()
    out_d = nc.dram_tensor("msgT", [BPC, D, E], F32, kind="ExternalOutput").ap()

    with tile.TileContext(nc) as tc:
        with (
            tc.tile_pool(name="consts", bufs=1) as consts,
            tc.tile_pool(name="inp", bufs=2) as inp,
            tc.tile_pool(name="work", bufs=6) as work,
            tc.tile_pool(name="ps_src", bufs=1, space="PSUM") as ps_src,
            tc.tile_pool(name="ps_rep", bufs=6, space="PSUM") as ps_rep,
            tc.tile_pool(name="ps_out", bufs=1, space="PSUM") as ps_out,
        ):
            # constants: W2 chunks + selector tiles (loaded once)
            w2_t = []
            for c in range(NCHUNK):
                t = consts.tile([128, D], DT, tag=f"w2_{c}")
                nc.sync.dma_start(t[:], w2_d[c])
                w2_t.append(t)
            sel_t = []
            for g in range(NG):
                t = consts.tile([128, 128], DT, tag=f"sel_{g}")
                nc.sync.dma_start(t[:], sel_d[g])
                sel_t.append(t)

            def start_batch(b):
                """DMA loads + src gather matmuls for one batch; returns
                per-batch state (srcT2, bt2, mout)."""
                at0 = inp.tile([128, 2 * D], DT, tag="at0")
                nc.sync.dma_start(at0[:], atom2_d[b, 0:128, :])
                at1 = inp.tile([128, 2 * D], DT, tag="at1")
                nc.sync.dma_start(at1[:], atom2_d[b, 128:256, :])
                bt2 = inp.tile([2 * K, E], DT, tag="bt2")
                nc.sync.dma_start(bt2[:], bondT2_d[b])
                oh0 = inp.tile([128, E], DT, tag="oh0")
                nc.sync.dma_start(oh0[:], onehot_d[b, 0:128, :])
                oh1 = inp.tile([128, E], DT, tag="oh1")
                nc.sync.dma_start(oh1[:], onehot_d[b, 128:256, :])

                # srcT2[p=(h,j), e] = src[e, j] (duplicated over h)
                ps = ps_src.tile([128, E], F32, tag="ps_src")
                nc.tensor.matmul(ps[:], at0[:], oh0[:],
                                 start=True, stop=False)
                nc.tensor.matmul(ps[:], at1[:], oh1[:],
                                 start=False, stop=True)
                srcT2 = work.tile([128, E], DT, tag="srcT2")
                nc.scalar.activation(srcT2[:], ps[:],
                                     mybir.ActivationFunctionType.Copy)
                mout = ps_out.tile([D, E], F32, tag="mout")
                return srcT2, bt2, mout, b

            def do_rep(st, g):
                _, bt2, _, _ = st
                rep_e = ps_rep.tile([128, E], F32, tag="rep")
                nc.tensor.matmul(rep_e[:], sel_t[g][0:64, :],
                                 bt2[0:64, :],
                                 start=True, stop=True,
                                 tile_position=(0, 0))
                rep_o = ps_rep.tile([128, E], F32, tag="rep")
                nc.tensor.matmul(rep_o[:], sel_t[g][64:128, :],
                                 bt2[64:128, :],
                                 start=True, stop=True,
                                 tile_position=(64, 0))
                return rep_e, rep_o

            def do_pair(st, g, reps):
                srcT2, _, mout, _ = st
                rep_e, rep_o = reps
                for half, rep in ((0, rep_e), (1, rep_o)):
                    c = 2 * g + half
                    op_t = work.tile([128, E], DT, tag="op")
                    if c % 8 in (1, 4, 6):
                        # offload 12/32 Hadamards to GPSIMD (~2x slower
                        # than DVE; rep staged to SBUF via the idle ACT
                        # since Pool can't read PSUM)
                        rep_s = work.tile([128, E], F32, tag="repc")
                        nc.scalar.activation(
                            rep_s[:], rep[:],
                            mybir.ActivationFunctionType.Copy)
                        nc.gpsimd.tensor_mul(op_t[:], srcT2[:], rep_s[:])
                    else:
                        nc.vector.tensor_mul(op_t[:], srcT2[:], rep[:])
                    nc.tensor.matmul(mout[:], w2_t[c][:], op_t[:],
                                     start=(c == 0), stop=(c == NCHUNK - 1))

            def finish_batch(st):
                _, _, mout, b = st
                res = work.tile([D, E], F32, tag="res")
                nc.scalar.activation(res[:], mout[:],
                                     mybir.ActivationFunctionType.Copy)
                nc.sync.dma_start(out_d[b], res[:])

            def body(_=None):
                # staggered batches: next batch's DMAs, src-gather matmuls
                # and first rep pair are issued near the end of the current
                # batch's chunk loop so the PE/DVE never drain between
                # batches
                from collections import deque
                sts = {0: None}
                fifos = {}

                def launch(b):
                    st = start_batch(b)
                    f = deque()
                    f.append((0, do_rep(st, 0)))
                    f.append((1, do_rep(st, 1)))
                    sts[b] = st
                    fifos[b] = f

                launch(0)
                for b in range(BPC):
                    st = sts[b]
                    fifo = fifos[b]
                    for g in range(NG):
                        if g + 2 < NG:
                            fifo.append((g + 2, do_rep(st, g + 2)))
                        if g == NG - 3 and b + 1 < BPC:
                            launch(b + 1)
                        gg, reps = fifo.popleft()
                        do_pair(st, gg, reps)
                    finish_batch(st)

            if repeat == 1:
                body()
            else:
                tc.For_i_unrolled(0, repeat, 1, body, max_unroll=1)

    nc.compile()
    return nc


_CACHE = {}


def _get_nc(repeat=1, use_f32r=True):
    key = (repeat, use_f32r)
    if key not in _CACHE:
        _CACHE[key] = _build(repeat, use_f32r)
    return _CACHE[key]


def _prep_core_inputs(atom_state, bond_state, connectivity):
    """Host-side layout prep for one core's batch slice (no float math)."""
    bpc = atom_state.shape[0]
    atom2 = np.concatenate([atom_state, atom_state], axis=2)          # [b,A,2D]
    atom2 = np.ascontiguousarray(atom2, dtype=np.float32)
    bT = np.swapaxes(bond_state, 1, 2)                                # [b,K,E]
    bondT2 = np.concatenate([bT, bT], axis=1)                         # [b,2K,E]
    bondT2 = np.ascontiguousarray(bondT2, dtype=np.float32)
    idx = connectivity[:, :, 0].astype(np.int64)                      # [b,E]
    onehotT = (idx[:, None, :] == np.arange(A)[None, :, None])
    onehotT = np.ascontiguousarray(onehotT, dtype=np.float32)         # [b,A,E]
    assert atom2.shape == (bpc, A, 2 * D)
    return atom2, bondT2, onehotT


def _shared_inputs(bond_transform):
    w2 = bond_transform.reshape(K, D, D).transpose(0, 2, 1).reshape(K * D, D)
    w2 = np.ascontiguousarray(w2.reshape(NCHUNK, 128, D), dtype=np.float32)
    sel = np.zeros((NG, 128, 128), dtype=np.float32)
    for g in range(NG):
        for h in range(2):
            # rows 0:64  -> chunk 2g   : k = 4g + h
            sel[g, 4 * g + h, h * D:(h + 1) * D] = 1.0
            # rows 64:128-> chunk 2g+1 : k = 4g + 2 + h
            sel[g, 64 + 4 * g + 2 + h, h * D:(h + 1) * D] = 1.0
    return w2, sel


def kernel(atom_state, bond_state, connectivity, bond_transform,
           repeat=1, use_f32r=True):
    atom_state = np.asarray(atom_state, dtype=np.float32)
    bond_state = np.asarray(bond_state, dtype=np.float32)
    connectivity = np.asarray(connectivity)
    bond_transform = np.asarray(bond_transform, dtype=np.float32)

    nc = _get_nc(repeat, use_f32r)
    w2, sel = _shared_inputs(bond_transform)

    in_maps = []
    for m in range(NCORES):
        sl = slice(m * BPC, (m + 1) * BPC)
        atom2, bondT2, onehotT = _prep_core_inputs(
            atom_state[sl], bond_state[sl], connectivity[sl])
        in_maps.append({
            "atom2": atom2,
            "bondT2": bondT2,
            "onehotT": onehotT,
            "w2": w2,
            "sel": sel,
        })

    res = run_bass_kernel_spmd(nc, in_maps, list(range(NCORES)))

    out = np.empty((B, E, D), dtype=np.float32)
    for m in range(NCORES):
        msgT = res.results[m]["msgT"]                                  # [BPC,D,E]
        out[m * BPC:(m + 1) * BPC] = np.swapaxes(msgT, 1, 2)
    return out


if __name__ == "__main__":
    rng = np.random.default_rng(0)
    atom = rng.standard_normal((B, A, D)).astype(np.float32)
    bond = rng.standard_normal((B, E, K)).astype(np.float32)
    conn = rng.integers(0, A, size=(B, E, 2)).astype(np.int64)
    bt = rng.standard_normal((K, D * D)).astype(np.float32) * 0.01
    out = kernel(atom, bond, conn, bt)
    print("out", out.shape, out.dtype, float(np.abs(out).max()))
